# revision 9
# baseline (speedup 1.0000x reference)
"""Trainium2 Bass kernel for nn_AttnReadout (segment attention readout).

Computation (reference):
    anchor[b]  = mean of ifeat rows in segment b                  [B, D]
    e[i]       = sigmoid(ifeat @ Wu.T + (anchor @ Wv.T + bv)[seg]) @ we
    alpha      = segment_softmax(e)
    rst[b]     = sum_i alpha[i] * ifeat[i]                        [B, D]
    out        = concat([rst, anchor], axis=1)                    [B, 2D]

Sharding: 2048 segments -> 8 cores x 2 windows of 128 contiguous segments.
Nodes (sorted by segment) are padded per-window to T_W tiles of 128 rows.

Design (per 128-node tile, engine assignment):
  - fc_u projection: fp8 (e4m3) DoubleRow matmul over both 128-deep
    k-subtiles in one instruction (128 effective cols vs 512 bf16); the
    transposed ifeat copy ships from HBM in fp8, halving that DMA stream.
    fp8 on the logits path only measures 8.9e-3 rel err (gate 2e-2). [PE]
  - per-node anchor gather (+fv): fp8 DoubleRow matmul with stationary
    [ohT_t, ohT_{t+1}] pairs and moving [fv8, 0]; the zero moving slot
    nullifies the second (wrong) one-hot so no per-tile memsets are
    needed.  Last tile of each 4-batch uses a plain fp8 matmul. [PE]
  - one-hot generation via iota==seg tensor_scalar (4x DVE mode).  NOT on
    GPSIMD: pool is_equal measured ~1.6us/op on HW (15x the cost model);
    a pool version of this kernel ran 297us vs 108us. [DVE]
  - one-hot transposes batched 4x into one PSUM bank; one PSUM->SBUF
    fp8 copy per batch, alternating DVE / ACT to balance load.
  - sigmoid batched 2 tiles per ACT instruction (amortizes the ~185ns
    ACT access-latency overhead). [ACT]
  - e-dot via scalar_tensor_tensor accumulate. [DVE]
  - z = exp(e) = sigmoid(e)/sigmoid(-e) per T_W/4 chunk: stays on the
    sigmoid ACT table (a table swap costs ~1.3us). [ACT + DVE]
  - anchor/wsum segment sums stay bf16 one-hot matmuls: fp8 values fail
    the 2e-2 gate (2.8e-2 measured in numpy). [PE]
  - software-pipelined emission: pass-1 of window w+1 interleaves into
    pass-2 of window w tile-by-tile; one-hot transposes run one 4-batch
    ahead of the gathers that consume them; weighted sums trail the
    logits stage by one z-chunk.

Rejected variants (all HW-measured slower): one-hot gen on GPSIMD
(297us); 4 windows of 64 segments + e-dot as PE matvec in [d, node]
orientation (224us - K=64 matmuls and tiny-moving matvecs are far below
the cost model on real HW).
"""

import numpy as np
import ml_dtypes

N = 102400
D = 256
B = 2048
N_CORES = 8
W_PER_CORE = 2
N_WINDOWS = N_CORES * W_PER_CORE  # 16
SEGS_PER_WINDOW = B // N_WINDOWS  # 128
P = 128
BF = ml_dtypes.bfloat16
F8 = ml_dtypes.float8_e4m3


def _apply_tile_patch():
    """Split TileContext's multi-wait tail drain into single-wait drains
    (this walrus build rejects >1 sync wait on a Drain instruction)."""
    import concourse.tile as tile_mod
    from concourse.vector_clock import ScopedClock

    if getattr(tile_mod.TileContext, "_drain_wait_split_patch", False):
        return

    def _patched(self, tick_clock, wait_clock):
        nc = self.nc
        drain_inst = nc.sync.drain()
        wait_clock.add_sem_waits(
            drain_inst.ins, ScopedClock({None: tick_clock.global_clock})
        )
        si = drain_inst.ins.sync_info
        waits = list(si.on_wait) if si is not None else []
        if len(waits) > 1:
            SyncInfo = type(si)
            drain_inst.ins.sync_info = SyncInfo(
                on_wait=[waits[0]], on_update=list(si.on_update)
            )
            for w in waits[1:]:
                extra = nc.sync.drain()
                extra.ins.sync_info = SyncInfo(on_wait=[w], on_update=[])

        nc.all_engine_barrier()
        assert self.sems is not None
        popped = nc._tile_sem_poison_stack.pop()
        assert popped is self._sem_poison
        nc.clear_and_free_semaphores(list(self.sems.allocated().values()))
        nc.all_engine_barrier()

    tile_mod.TileContext._drain_and_barrier = _patched
    tile_mod.TileContext._drain_wait_split_patch = True


def _split_sync_waits(nc, limit=1):
    """Split >limit sync waits per instruction into preceding single-wait
    EventSemaphore carriers on the same engine (walrus build limit)."""
    import concourse.mybir as mybir

    n_new = 0
    for _, bassbb in nc.bb_map.items():
        insts = bassbb.bb.instructions  # live list
        snapshot = list(insts)
        offset = 0
        for pos, inst in enumerate(snapshot):
            si = getattr(inst, "sync_info", None)
            if si is None:
                continue
            waits = list(si.on_wait)
            if len(waits) <= limit:
                continue
            SyncInfo = type(si)
            inst.sync_info = SyncInfo(
                on_wait=waits[:limit], on_update=list(si.on_update))
            carriers = []
            for w in waits[limit:]:
                c = mybir.InstEventSemaphore(
                    name=f"WSPLIT-{nc.next_id()}", ins=[], outs=[])
                c.engine = inst.engine
                c.sync_info = SyncInfo(on_wait=[w], on_update=[])
                carriers.append(c)
            insts[pos + offset:pos + offset] = carriers
            offset += len(carriers)
            n_new += len(carriers)
    return n_new


def _build(T_W, repeat=1, loop_repeat=None):
    """Build the single-core SPMD Bass program; T_W must be a multiple of 4."""
    import contextlib
    import concourse.bass as bass
    import concourse.mybir as mybir
    from concourse.tile import TileContext

    _apply_tile_patch()

    f32 = mybir.dt.float32
    bf16 = mybir.dt.bfloat16
    fp8 = mybir.dt.float8e4
    Alu = mybir.AluOpType
    Act = mybir.ActivationFunctionType
    DR = mybir.MatmulPerfMode.DoubleRow

    assert T_W % 4 == 0
    CHD = T_W // 2          # tiles per DMA chunk (2 chunks per window)
    CHZ = T_W // 4          # tiles per z chunk (4 chunks per window)
    NT = W_PER_CORE * T_W

    nc = bass.Bass("TRN2", num_devices=N_CORES)

    nat_dram = nc.dram_tensor("natp", [P, NT, D + 1], bf16, kind="ExternalInput")
    ifT_dram = nc.dram_tensor("iftp", [P, NT, 2, P], fp8, kind="ExternalInput")
    seg_dram = nc.dram_tensor("segp", [P, NT], f32, kind="ExternalInput")
    wu8_dram = nc.dram_tensor("wu8", [2, P, D], fp8, kind="ExternalInput")
    wvT_dram = nc.dram_tensor("wvT", [2, P, D], bf16, kind="ExternalInput")
    web_dram = nc.dram_tensor("web", [P, 2 * D], bf16, kind="ExternalInput")
    bvb_dram = nc.dram_tensor("bvb", [P, D], f32, kind="ExternalInput")
    idb_dram = nc.dram_tensor("idb", [P, P], bf16, kind="ExternalInput")
    iota_dram = nc.dram_tensor("iota", [P, P], bf16, kind="ExternalInput")
    out_dram = nc.dram_tensor("out", [W_PER_CORE, P, 2 * D], f32,
                              kind="ExternalOutput")

    with TileContext(nc) as tc:
        with contextlib.ExitStack() as ctx:
            const_pool = ctx.enter_context(tc.tile_pool(name="const", bufs=1))
            nat_pool = ctx.enter_context(tc.tile_pool(name="nat", bufs=4))
            ifT_pool = ctx.enter_context(tc.tile_pool(name="ifT", bufs=4))
            ohw_pool = ctx.enter_context(tc.tile_pool(name="ohw", bufs=2))
            ohT_pool = ctx.enter_context(tc.tile_pool(name="ohT", bufs=3))
            s_pool = ctx.enter_context(tc.tile_pool(name="s", bufs=3))
            prod_pool = ctx.enter_context(tc.tile_pool(name="prod", bufs=3))
            ohz_pool = ctx.enter_context(tc.tile_pool(name="ohz", bufs=4))
            ew_pool = ctx.enter_context(tc.tile_pool(name="ew", bufs=2))
            zch_pool = ctx.enter_context(tc.tile_pool(name="zch", bufs=4))
            wnd_pool = ctx.enter_context(tc.tile_pool(name="wnd", bufs=2))
            col_pool = ctx.enter_context(tc.tile_pool(name="col", bufs=12))
            anchor_ps_pool = ctx.enter_context(
                tc.tile_pool(name="anchor_ps", bufs=1, space="PSUM"))
            wsum_ps_pool = ctx.enter_context(
                tc.tile_pool(name="wsum_ps", bufs=1, space="PSUM"))
            trb_ps_pool = ctx.enter_context(
                tc.tile_pool(name="trb_ps", bufs=2, space="PSUM"))
            s_ps_pool = ctx.enter_context(
                tc.tile_pool(name="s_ps", bufs=3, space="PSUM"))
            fv_ps_pool = ctx.enter_context(
                tc.tile_pool(name="fv_ps", bufs=1, space="PSUM"))

            # constants
            wu8_sb = const_pool.tile([P, 2, D], fp8, name="wu8_sb", tag="wu8")
            nc.sync.dma_start(wu8_sb[:], wu8_dram[:].rearrange("k p d -> p k d"))
            wvT_sb = const_pool.tile([P, 2, D], bf16, name="wvT_sb", tag="wvT")
            nc.sync.dma_start(wvT_sb[:], wvT_dram[:].rearrange("k p d -> p k d"))
            web_sb = const_pool.tile([P, 2 * D], bf16, name="web_sb", tag="web")
            nc.sync.dma_start(web_sb[:], web_dram[:])
            bvb_sb = const_pool.tile([P, D], f32, name="bvb_sb", tag="bvb")
            nc.sync.dma_start(bvb_sb[:], bvb_dram[:])
            idb_sb = const_pool.tile([P, P], bf16, name="idb_sb", tag="idb")
            nc.sync.dma_start(idb_sb[:], idb_dram[:])
            iota_sb = const_pool.tile([P, P], bf16, name="iota_sb", tag="iota")
            nc.sync.dma_start(iota_sb[:], iota_dram[:])
            seg_sb = const_pool.tile([P, NT], f32, name="seg_sb", tag="seg")
            nc.sync.dma_start(seg_sb[:], seg_dram[:])

            def body(rep):
                # per-window state kept across generator stages
                st = [dict() for _ in range(W_PER_CORE)]

                def p1_loads(w):
                    """Issue the window's chunk loads."""
                    s = st[w]
                    nat_ch, ifT_ch = {}, {}
                    for cl in range(2):
                        c = 2 * w + cl
                        natc = nat_pool.tile([P, CHD, D + 1], bf16,
                                             name=f"natc{rep}_{c}", tag="natc")
                        nc.sync.dma_start(
                            natc[:], nat_dram[:, c * CHD:(c + 1) * CHD, :])
                        nat_ch[cl] = natc
                        iftc = ifT_pool.tile([P, CHD, 2, P], fp8,
                                             name=f"iftc{rep}_{c}", tag="iftc")
                        nc.sync.dma_start(
                            iftc[:], ifT_dram[:, c * CHD:(c + 1) * CHD, :, :])
                        ifT_ch[cl] = iftc
                    s["nat"] = lambda t: nat_ch[t // CHD][:, t % CHD, :]
                    s["ifT"] = lambda t: ifT_ch[t // CHD][:, t % CHD, :, :]

                def p1(w):
                    """Pass 1: loads, one-hot gen, anchor accumulation."""
                    p1_loads(w)
                    s = st[w]
                    ohw = ohw_pool.tile([P, T_W, P], bf16,
                                        name=f"ohw{rep}_{w}", tag="ohw")
                    s["ohw"] = ohw
                    anchor_ps = anchor_ps_pool.tile(
                        [P, D + 1], f32, name=f"anc{rep}_{w}", tag="anchor_ps")
                    s["anchor_ps"] = anchor_ps
                    for t in range(T_W):
                        g = w * T_W + t
                        nc.vector.tensor_scalar(
                            ohw[:, t, :], iota_sb[:], seg_sb[:, g:g + 1], None,
                            Alu.is_equal)
                        nc.tensor.matmul(anchor_ps[:], ohw[:, t, :],
                                         s["nat"](t), start=(t == 0),
                                         stop=(t == T_W - 1))
                        yield

                def mid(w):
                    """Anchor normalization, fv = anchor @ Wv.T + bv, fvz."""
                    s = st[w]
                    anchor_ps = s["anchor_ps"]
                    cnt = col_pool.tile([P, 1], f32, name=f"cnt{rep}_{w}",
                                        tag="col")
                    nc.vector.tensor_scalar(cnt[:], anchor_ps[:, D:D + 1], 1.0,
                                            None, Alu.max)
                    rcnt = col_pool.tile([P, 1], f32, name=f"rcnt{rep}_{w}",
                                         tag="col")
                    nc.vector.reciprocal(rcnt[:], cnt[:])
                    out_sb = wnd_pool.tile([P, 2 * D], f32, name=f"osb{rep}_{w}",
                                           tag="out_sb")
                    s["out_sb"] = out_sb
                    nc.vector.tensor_scalar(out_sb[:, D:2 * D],
                                            anchor_ps[:, 0:D], rcnt[:], None,
                                            Alu.mult)
                    anchor_bf = wnd_pool.tile([P, D], bf16,
                                              name=f"anbf{rep}_{w}", tag="anbf")
                    nc.vector.tensor_scalar(anchor_bf[:], anchor_ps[:, 0:D],
                                            rcnt[:], None, Alu.mult)
                    trb = trb_ps_pool.tile([P, 4, P], bf16,
                                           name=f"atr{rep}_{w}", tag="trb")
                    for db in range(2):
                        nc.tensor.transpose(trb[:, db, :],
                                            anchor_bf[:, db * P:(db + 1) * P],
                                            idb_sb[:])
                    anchT = wnd_pool.tile([P, 2, P], bf16,
                                          name=f"anchT{rep}_{w}", tag="anchT")
                    nc.vector.tensor_copy(anchT[:], trb[:, 0:2, :])
                    fv_ps = fv_ps_pool.tile([P, D], f32, name=f"fv{rep}_{w}",
                                            tag="fv_ps")
                    for db in range(2):
                        nc.tensor.matmul(fv_ps[:], anchT[:, db, :],
                                         wvT_sb[:, db, :], start=(db == 0),
                                         stop=(db == 1))
                    fvz = wnd_pool.tile([P, 2, D], fp8, name=f"fvz{rep}_{w}",
                                        tag="fvz")
                    s["fvz"] = fvz
                    nc.vector.tensor_tensor(fvz[:, 0, :], fv_ps[:], bvb_sb[:],
                                            Alu.add)
                    nc.vector.memset(fvz[:, 1, :], 0.0)

                def p2(w):
                    """Pass 2: logits, segment softmax, weighted segment sum."""
                    s = st[w]
                    ohw, fvz = s["ohw"], s["fvz"]
                    wsum_ps = wsum_ps_pool.tile(
                        [P, D + 1], f32, name=f"ws{rep}_{w}", tag="wsum_ps")
                    e_win = ew_pool.tile([P, T_W], f32, name=f"ew{rep}_{w}",
                                         tag="e_win")
                    z_win = ew_pool.tile([P, T_W], f32, name=f"zw{rep}_{w}",
                                         tag="z_win")
                    ohT = {}

                    def emit_trb_batch(k):
                        # transpose tiles 4k..4k+3 into one PSUM bank, one copy
                        trb = trb_ps_pool.tile([P, 4, P], bf16,
                                               name=f"trb{rep}_{w}_{k}",
                                               tag="trb")
                        for j in range(4):
                            nc.tensor.transpose(trb[:, j, :],
                                                ohw[:, 4 * k + j, :], idb_sb[:])
                        oh4 = ohT_pool.tile([P, 4, P], fp8,
                                            name=f"ohT{rep}_{w}_{k}", tag="ohT")
                        nc.scalar.copy(oh4[:], trb[:])
                        ohT[k] = oh4

                    emit_trb_batch(0)
                    n_wsum = 0  # tiles whose ohz+wsum have been emitted

                    def emit_wsum_tile():
                        nonlocal n_wsum
                        t = n_wsum
                        ohz = ohz_pool.tile([P, P], bf16,
                                            name=f"ohz{rep}_{w}_{t}", tag="ohz")
                        nc.vector.tensor_scalar(ohz[:], ohw[:, t, :],
                                                z_win[:, t:t + 1], None,
                                                Alu.mult)
                        nc.tensor.matmul(wsum_ps[:], ohz[:], s["nat"](t),
                                         start=(t == 0), stop=(t == T_W - 1))
                        n_wsum += 1

                    def emit_z_chunk(c):
                        # z = exp(e) = sigmoid(e)/sigmoid(-e): stays on the
                        # sigmoid ACT table (no table swaps).
                        c0, c1 = c * CHZ, (c + 1) * CHZ
                        sp = zch_pool.tile([P, CHZ], f32,
                                           name=f"sp{rep}_{w}_{c}", tag="zch")
                        nc.scalar.activation(sp[:], e_win[:, c0:c1],
                                             Act.Sigmoid)
                        sn = zch_pool.tile([P, CHZ], f32,
                                           name=f"sn{rep}_{w}_{c}", tag="zch")
                        nc.scalar.activation(sn[:], e_win[:, c0:c1],
                                             Act.Sigmoid, scale=-1.0)
                        rn = zch_pool.tile([P, CHZ], f32,
                                           name=f"rn{rep}_{w}_{c}", tag="zch")
                        nc.vector.reciprocal(rn[:], sn[:])
                        nc.vector.tensor_tensor(z_win[:, c0:c1], sp[:],
                                                rn[:], Alu.mult)

                    s_ps = None
                    zc = 0  # z chunks emitted
                    for t in range(T_W):
                        if t % 4 == 0 and t + 4 < T_W:
                            emit_trb_batch(t // 4 + 1)
                        if t % 2 == 0:
                            s_ps = s_ps_pool.tile([P, 2 * D], f32,
                                                  name=f"sps{rep}_{w}_{t}",
                                                  tag="s_ps")
                        sl = slice((t % 2) * D, (t % 2) * D + D)
                        nc.tensor.matmul(s_ps[:, sl], s["ifT"](t), wu8_sb[:],
                                         start=True, stop=False, perf_mode=DR)
                        k, j = t // 4, t % 4
                        if j < 3:
                            nc.tensor.matmul(s_ps[:, sl], ohT[k][:, j:j + 2, :],
                                             fvz[:], start=False, stop=True,
                                             perf_mode=DR)
                        else:
                            nc.tensor.matmul(s_ps[:, sl], ohT[k][:, 3, :],
                                             fvz[:, 0, :], start=False,
                                             stop=True)
                        if t % 2 == 1:
                            s_sb = s_pool.tile([P, 2 * D], bf16,
                                               name=f"ssb{rep}_{w}_{t}",
                                               tag="s_sb")
                            nc.scalar.activation(s_sb[:], s_ps[:], Act.Sigmoid)
                            # e-dot as 2x-mode TT multiply (both tiles at
                            # once) + 4x-mode TS accumulates: ~37ns/tile
                            # cheaper on DVE than the mode-less stt
                            prod = prod_pool.tile(
                                [P, 2 * D], bf16, name=f"pr{rep}_{w}_{t}",
                                tag="prod")
                            nc.vector.tensor_tensor(prod[:], s_sb[:],
                                                    web_sb[:], Alu.mult)
                            for tt in (t - 1, t):
                                ssl = slice((tt % 2) * D, (tt % 2) * D + D)
                                junk = s_pool.tile(
                                    [P, D], bf16, name=f"jk{rep}_{w}_{tt}",
                                    tag="junk")
                                nc.vector.tensor_scalar(
                                    junk[:], prod[:, ssl], 1.0, None,
                                    Alu.mult, Alu.add,
                                    accum_out=e_win[:, tt:tt + 1])
                        # emit z chunks whose e columns are all accumulated
                        stt_done = t + 1 if t % 2 == 1 else t - 1
                        while zc < 4 and (zc + 1) * CHZ <= stt_done:
                            emit_z_chunk(zc)
                            zc += 1
                        # trail the logits stage with z-gated weighted sums
                        if n_wsum < zc * CHZ and n_wsum <= t - CHZ:
                            emit_wsum_tile()
                        yield
                    while n_wsum < T_W:
                        emit_wsum_tile()
                        if n_wsum % 4 == 0:
                            yield

                    den = col_pool.tile([P, 1], f32, name=f"den{rep}_{w}",
                                        tag="col")
                    nc.vector.tensor_scalar(den[:], wsum_ps[:, D:D + 1], 1e-30,
                                            None, Alu.max)
                    rden = col_pool.tile([P, 1], f32, name=f"rden{rep}_{w}",
                                         tag="col")
                    nc.vector.reciprocal(rden[:], den[:])
                    out_sb = s["out_sb"]
                    nc.vector.tensor_scalar(out_sb[:, 0:D], wsum_ps[:, 0:D],
                                            rden[:], None, Alu.mult)
                    nc.sync.dma_start(out_dram[w], out_sb[:])

                # software pipeline: p1(0); [p2(w) x p1(w+1)]; p2 tail
                for _ in p1(0):
                    pass
                for w in range(W_PER_CORE):
                    mid(w)
                    nxt = p1(w + 1) if w + 1 < W_PER_CORE else None
                    for _ in p2(w):
                        if nxt is not None:
                            next(nxt, None)
                    if nxt is not None:
                        for _ in nxt:
                            pass

            if loop_repeat is not None:
                with tc.For_i(0, loop_repeat, 1):
                    body("L")
            else:
                for rep in range(repeat):
                    body(rep)

    return nc


def _prepare(ifeat, Wu, Wv, bv, we, seg_ids):
    """Host-side shard + pad + layout. Returns (T_W, in_maps)."""
    ifeat = np.asarray(ifeat, dtype=np.float32)
    Wu = np.asarray(Wu, dtype=np.float32)
    Wv = np.asarray(Wv, dtype=np.float32)
    bv = np.asarray(bv, dtype=np.float32)
    we = np.asarray(we, dtype=np.float32)
    seg_ids = np.asarray(seg_ids)

    bounds = np.searchsorted(
        seg_ids, np.arange(0, B + 1, SEGS_PER_WINDOW), side="left")
    n_w = np.diff(bounds)
    T_W = max(4, int(-(-int(n_w.max()) // P)))
    T_W = ((T_W + 3) // 4) * 4
    NT = W_PER_CORE * T_W

    wu8 = np.ascontiguousarray(Wu.T).reshape(2, P, D).astype(F8)
    wvT = np.ascontiguousarray(Wv.T).reshape(2, P, D).astype(BF)
    web = np.tile(we, (P, 2)).astype(BF)
    bvb = np.tile(bv, (P, 1)).astype(np.float32)
    idb = np.eye(P, dtype=BF)
    iota = np.tile(np.arange(P, dtype=np.float32), (P, 1)).astype(BF)

    in_maps = []
    for c in range(N_CORES):
        nat = np.zeros((NT * P, D + 1), dtype=np.float32)
        nat[:, D] = 1.0
        seg = np.full((NT * P,), 500.0, dtype=np.float32)
        for wl in range(W_PER_CORE):
            w = c * W_PER_CORE + wl
            lo, hi = bounds[w], bounds[w + 1]
            base = wl * T_W * P
            nat[base:base + (hi - lo), 0:D] = ifeat[lo:hi]
            seg[base:base + (hi - lo)] = (
                seg_ids[lo:hi].astype(np.float32) - w * SEGS_PER_WINDOW)
        natb = nat.astype(BF).reshape(NT, P, D + 1)
        natp = np.ascontiguousarray(natb.transpose(1, 0, 2))      # [P, NT, 257]
        x = nat[:, 0:D].astype(F8).reshape(NT, P, 2, P)           # [g,i,kb,d]
        iftp = np.ascontiguousarray(x.transpose(3, 0, 2, 1))      # [d, g, kb, i]
        segp = np.ascontiguousarray(seg.reshape(NT, P).T)         # [P, NT]
        in_maps.append({
            "natp": natp, "iftp": iftp, "segp": segp,
            "wu8": wu8, "wvT": wvT, "web": web, "bvb": bvb,
            "idb": idb, "iota": iota,
        })
    return T_W, in_maps


_LAST = {}


def _run(ifeat, Wu, Wv, bv, we, seg_ids, trace=False):
    from concourse.bass_utils import run_bass_kernel_spmd

    T_W, in_maps = _prepare(ifeat, Wu, Wv, bv, we, seg_ids)
    nc = _build(T_W)
    _split_sync_waits(nc)
    res = run_bass_kernel_spmd(nc, in_maps, list(range(N_CORES)), trace=trace)
    _LAST["res"] = res
    _LAST["T_W"] = T_W
    _LAST["nc"] = nc
    _LAST["in_maps"] = in_maps

    out = np.empty((B, 2 * D), dtype=np.float32)
    for c in range(N_CORES):
        core_out = res.results[c]["out"]  # [W_PER_CORE, P, 2D]
        for wl in range(W_PER_CORE):
            w = c * W_PER_CORE + wl
            out[w * SEGS_PER_WINDOW:(w + 1) * SEGS_PER_WINDOW, :] = core_out[wl]
    return out


def kernel(ifeat, Wu, Wv, bv, we, seg_ids):
    return _run(ifeat, Wu, Wv, bv, we, seg_ids, trace=False)


# revision 10
# speedup vs baseline: 1.4009x; 1.4009x over previous
"""Trainium2 Bass kernel for nn_AttnReadout (segment attention readout).

Computation (reference):
    anchor[b]  = mean of ifeat rows in segment b                  [B, D]
    e[i]       = sigmoid(ifeat @ Wu.T + (anchor @ Wv.T + bv)[seg]) @ we
    alpha      = segment_softmax(e)
    rst[b]     = sum_i alpha[i] * ifeat[i]                        [B, D]
    out        = concat([rst, anchor], axis=1)                    [B, 2D]

Sharding: 2048 segments -> 8 cores x 2 windows of 128 contiguous segments.
Nodes (sorted by segment) are padded per-window to T_W tiles of 128 rows.

Design (per 128-node tile, engine assignment):
  - fc_u projection: fp8 (e4m3) DoubleRow matmul over both 128-deep
    k-subtiles in one instruction (128 effective cols vs 512 bf16); the
    transposed ifeat copy ships from HBM in fp8, halving that DMA stream.
    fp8 on the logits path only measures 8.9e-3 rel err (gate 2e-2). [PE]
  - per-node anchor gather (+fv): fp8 DoubleRow matmul with stationary
    [ohT_t, ohT_{t+1}] pairs and moving [fv8, 0]; the zero moving slot
    nullifies the second (wrong) one-hot so no per-tile memsets are
    needed.  Last tile of each 4-batch uses a plain fp8 matmul. [PE]
  - one-hot generation via iota==seg tensor_scalar (4x DVE mode).  NOT on
    GPSIMD: pool is_equal measured ~1.6us/op on HW (15x the cost model);
    a pool version of this kernel ran 297us vs 108us. [DVE]
  - one-hot transposes batched 4x into one PSUM bank; one PSUM->SBUF
    fp8 copy per batch, alternating DVE / ACT to balance load.
  - sigmoid batched 2 tiles per ACT instruction (amortizes the ~185ns
    ACT access-latency overhead). [ACT]
  - e-dot via scalar_tensor_tensor accumulate. [DVE]
  - z = exp(e) = sigmoid(e)/sigmoid(-e) per T_W/4 chunk: stays on the
    sigmoid ACT table (a table swap costs ~1.3us). [ACT + DVE]
  - anchor/wsum segment sums stay bf16 one-hot matmuls: fp8 values fail
    the 2e-2 gate (2.8e-2 measured in numpy). [PE]
  - software-pipelined emission: pass-1 of window w+1 interleaves into
    pass-2 of window w tile-by-tile; one-hot transposes run one 4-batch
    ahead of the gathers that consume them; weighted sums trail the
    logits stage by one z-chunk.

Rejected variants (all HW-measured slower): one-hot gen on GPSIMD
(297us); 4 windows of 64 segments + e-dot as PE matvec in [d, node]
orientation (224us - K=64 matmuls and tiny-moving matvecs are far below
the cost model on real HW).
"""

import numpy as np
import ml_dtypes

N = 102400
D = 256
B = 2048
N_CORES = 8
W_PER_CORE = 2
N_WINDOWS = N_CORES * W_PER_CORE  # 16
SEGS_PER_WINDOW = B // N_WINDOWS  # 128
P = 128
BF = ml_dtypes.bfloat16
F8 = ml_dtypes.float8_e4m3


def _apply_tile_patch():
    """Split TileContext's multi-wait tail drain into single-wait drains
    (this walrus build rejects >1 sync wait on a Drain instruction)."""
    import concourse.tile as tile_mod
    from concourse.vector_clock import ScopedClock

    if getattr(tile_mod.TileContext, "_drain_wait_split_patch", False):
        return

    def _patched(self, tick_clock, wait_clock):
        nc = self.nc
        drain_inst = nc.sync.drain()
        wait_clock.add_sem_waits(
            drain_inst.ins, ScopedClock({None: tick_clock.global_clock})
        )
        si = drain_inst.ins.sync_info
        waits = list(si.on_wait) if si is not None else []
        if len(waits) > 1:
            SyncInfo = type(si)
            drain_inst.ins.sync_info = SyncInfo(
                on_wait=[waits[0]], on_update=list(si.on_update)
            )
            for w in waits[1:]:
                extra = nc.sync.drain()
                extra.ins.sync_info = SyncInfo(on_wait=[w], on_update=[])

        nc.all_engine_barrier()
        assert self.sems is not None
        popped = nc._tile_sem_poison_stack.pop()
        assert popped is self._sem_poison
        nc.clear_and_free_semaphores(list(self.sems.allocated().values()))
        nc.all_engine_barrier()

    tile_mod.TileContext._drain_and_barrier = _patched
    tile_mod.TileContext._drain_wait_split_patch = True


def _split_sync_waits(nc, limit=1):
    """Split >limit sync waits per instruction into preceding single-wait
    EventSemaphore carriers on the same engine (walrus build limit)."""
    import concourse.mybir as mybir

    n_new = 0
    for _, bassbb in nc.bb_map.items():
        insts = bassbb.bb.instructions  # live list
        snapshot = list(insts)
        offset = 0
        for pos, inst in enumerate(snapshot):
            si = getattr(inst, "sync_info", None)
            if si is None:
                continue
            waits = list(si.on_wait)
            if len(waits) <= limit:
                continue
            SyncInfo = type(si)
            inst.sync_info = SyncInfo(
                on_wait=waits[:limit], on_update=list(si.on_update))
            carriers = []
            for w in waits[limit:]:
                c = mybir.InstEventSemaphore(
                    name=f"WSPLIT-{nc.next_id()}", ins=[], outs=[])
                c.engine = inst.engine
                c.sync_info = SyncInfo(on_wait=[w], on_update=[])
                carriers.append(c)
            insts[pos + offset:pos + offset] = carriers
            offset += len(carriers)
            n_new += len(carriers)
    return n_new


def _build(T_W, repeat=1, loop_repeat=None):
    """Build the single-core SPMD Bass program; T_W must be a multiple of 4."""
    import contextlib
    import concourse.bass as bass
    import concourse.mybir as mybir
    from concourse.tile import TileContext

    _apply_tile_patch()

    f32 = mybir.dt.float32
    bf16 = mybir.dt.bfloat16
    fp8 = mybir.dt.float8e4
    Alu = mybir.AluOpType
    Act = mybir.ActivationFunctionType
    DR = mybir.MatmulPerfMode.DoubleRow

    assert T_W % 4 == 0
    CHD = T_W // 2          # tiles per DMA chunk (2 chunks per window)
    CHZ = T_W // 4          # tiles per z chunk (4 chunks per window)
    NT = W_PER_CORE * T_W

    nc = bass.Bass("TRN2", num_devices=N_CORES)

    nat_dram = nc.dram_tensor("natp", [P, NT, D + 1], bf16, kind="ExternalInput")
    ifT_dram = nc.dram_tensor("iftp", [P, NT, 2, P], fp8, kind="ExternalInput")
    seg_dram = nc.dram_tensor("segp", [P, NT], f32, kind="ExternalInput")
    wu8_dram = nc.dram_tensor("wu8", [2, P, D], fp8, kind="ExternalInput")
    wvT_dram = nc.dram_tensor("wvT", [2, P, D], bf16, kind="ExternalInput")
    web_dram = nc.dram_tensor("web", [P, D], bf16, kind="ExternalInput")
    bvb_dram = nc.dram_tensor("bvb", [P, D], f32, kind="ExternalInput")
    idb_dram = nc.dram_tensor("idb", [P, P], bf16, kind="ExternalInput")
    iota_dram = nc.dram_tensor("iota", [P, P], bf16, kind="ExternalInput")
    out_dram = nc.dram_tensor("out", [W_PER_CORE, P, 2 * D], f32,
                              kind="ExternalOutput")

    with TileContext(nc) as tc:
        with contextlib.ExitStack() as ctx:
            const_pool = ctx.enter_context(tc.tile_pool(name="const", bufs=1))
            nat_pool = ctx.enter_context(tc.tile_pool(name="nat", bufs=4))
            ifT_pool = ctx.enter_context(tc.tile_pool(name="ifT", bufs=4))
            ohw_pool = ctx.enter_context(tc.tile_pool(name="ohw", bufs=2))
            ohT_pool = ctx.enter_context(tc.tile_pool(name="ohT", bufs=3))
            s_pool = ctx.enter_context(tc.tile_pool(name="s", bufs=3))
            prod_pool = ctx.enter_context(tc.tile_pool(name="prod", bufs=3))
            ohz_pool = ctx.enter_context(tc.tile_pool(name="ohz", bufs=4))
            ew_pool = ctx.enter_context(tc.tile_pool(name="ew", bufs=2))
            zch_pool = ctx.enter_context(tc.tile_pool(name="zch", bufs=4))
            wnd_pool = ctx.enter_context(tc.tile_pool(name="wnd", bufs=2))
            col_pool = ctx.enter_context(tc.tile_pool(name="col", bufs=12))
            anchor_ps_pool = ctx.enter_context(
                tc.tile_pool(name="anchor_ps", bufs=1, space="PSUM"))
            wsum_ps_pool = ctx.enter_context(
                tc.tile_pool(name="wsum_ps", bufs=1, space="PSUM"))
            trb_ps_pool = ctx.enter_context(
                tc.tile_pool(name="trb_ps", bufs=2, space="PSUM"))
            s_ps_pool = ctx.enter_context(
                tc.tile_pool(name="s_ps", bufs=3, space="PSUM"))
            fv_ps_pool = ctx.enter_context(
                tc.tile_pool(name="fv_ps", bufs=1, space="PSUM"))

            # constants
            wu8_sb = const_pool.tile([P, 2, D], fp8, name="wu8_sb", tag="wu8")
            nc.sync.dma_start(wu8_sb[:], wu8_dram[:].rearrange("k p d -> p k d"))
            wvT_sb = const_pool.tile([P, 2, D], bf16, name="wvT_sb", tag="wvT")
            nc.sync.dma_start(wvT_sb[:], wvT_dram[:].rearrange("k p d -> p k d"))
            web_sb = const_pool.tile([P, D], bf16, name="web_sb", tag="web")
            nc.sync.dma_start(web_sb[:], web_dram[:])
            bvb_sb = const_pool.tile([P, D], f32, name="bvb_sb", tag="bvb")
            nc.sync.dma_start(bvb_sb[:], bvb_dram[:])
            idb_sb = const_pool.tile([P, P], bf16, name="idb_sb", tag="idb")
            nc.sync.dma_start(idb_sb[:], idb_dram[:])
            iota_sb = const_pool.tile([P, P], bf16, name="iota_sb", tag="iota")
            nc.sync.dma_start(iota_sb[:], iota_dram[:])
            seg_sb = const_pool.tile([P, NT], f32, name="seg_sb", tag="seg")
            nc.sync.dma_start(seg_sb[:], seg_dram[:])

            def body(rep):
                # per-window state kept across generator stages
                st = [dict() for _ in range(W_PER_CORE)]

                def p1_loads(w):
                    """Issue the window's chunk loads."""
                    s = st[w]
                    nat_ch, ifT_ch = {}, {}
                    for cl in range(2):
                        c = 2 * w + cl
                        natc = nat_pool.tile([P, CHD, D + 1], bf16,
                                             name=f"natc{rep}_{c}", tag="natc")
                        nc.sync.dma_start(
                            natc[:], nat_dram[:, c * CHD:(c + 1) * CHD, :])
                        nat_ch[cl] = natc
                        iftc = ifT_pool.tile([P, CHD, 2, P], fp8,
                                             name=f"iftc{rep}_{c}", tag="iftc")
                        nc.sync.dma_start(
                            iftc[:], ifT_dram[:, c * CHD:(c + 1) * CHD, :, :])
                        ifT_ch[cl] = iftc
                    s["nat"] = lambda t: nat_ch[t // CHD][:, t % CHD, :]
                    s["ifT"] = lambda t: ifT_ch[t // CHD][:, t % CHD, :, :]

                def p1(w):
                    """Pass 1: loads, one-hot gen, anchor accumulation."""
                    p1_loads(w)
                    s = st[w]
                    ohw = ohw_pool.tile([P, T_W, P], bf16,
                                        name=f"ohw{rep}_{w}", tag="ohw")
                    s["ohw"] = ohw
                    anchor_ps = anchor_ps_pool.tile(
                        [P, D + 1], f32, name=f"anc{rep}_{w}", tag="anchor_ps")
                    s["anchor_ps"] = anchor_ps
                    for t in range(T_W):
                        g = w * T_W + t
                        nc.vector.tensor_scalar(
                            ohw[:, t, :], iota_sb[:], seg_sb[:, g:g + 1], None,
                            Alu.is_equal)
                        nc.tensor.matmul(anchor_ps[:], ohw[:, t, :],
                                         s["nat"](t), start=(t == 0),
                                         stop=(t == T_W - 1))
                        yield

                def mid(w):
                    """Anchor normalization, fv = anchor @ Wv.T + bv, fvz."""
                    s = st[w]
                    anchor_ps = s["anchor_ps"]
                    cnt = col_pool.tile([P, 1], f32, name=f"cnt{rep}_{w}",
                                        tag="col")
                    nc.vector.tensor_scalar(cnt[:], anchor_ps[:, D:D + 1], 1.0,
                                            None, Alu.max)
                    rcnt = col_pool.tile([P, 1], f32, name=f"rcnt{rep}_{w}",
                                         tag="col")
                    nc.vector.reciprocal(rcnt[:], cnt[:])
                    out_sb = wnd_pool.tile([P, 2 * D], f32, name=f"osb{rep}_{w}",
                                           tag="out_sb")
                    s["out_sb"] = out_sb
                    nc.vector.tensor_scalar(out_sb[:, D:2 * D],
                                            anchor_ps[:, 0:D], rcnt[:], None,
                                            Alu.mult)
                    anchor_bf = wnd_pool.tile([P, D], bf16,
                                              name=f"anbf{rep}_{w}", tag="anbf")
                    nc.vector.tensor_scalar(anchor_bf[:], anchor_ps[:, 0:D],
                                            rcnt[:], None, Alu.mult)
                    trb = trb_ps_pool.tile([P, 4, P], bf16,
                                           name=f"atr{rep}_{w}", tag="trb")
                    for db in range(2):
                        nc.tensor.transpose(trb[:, db, :],
                                            anchor_bf[:, db * P:(db + 1) * P],
                                            idb_sb[:])
                    anchT = wnd_pool.tile([P, 2, P], bf16,
                                          name=f"anchT{rep}_{w}", tag="anchT")
                    nc.vector.tensor_copy(anchT[:], trb[:, 0:2, :])
                    fv_ps = fv_ps_pool.tile([P, D], f32, name=f"fv{rep}_{w}",
                                            tag="fv_ps")
                    for db in range(2):
                        nc.tensor.matmul(fv_ps[:], anchT[:, db, :],
                                         wvT_sb[:, db, :], start=(db == 0),
                                         stop=(db == 1))
                    fvz = wnd_pool.tile([P, 2, D], fp8, name=f"fvz{rep}_{w}",
                                        tag="fvz")
                    s["fvz"] = fvz
                    nc.vector.tensor_tensor(fvz[:, 0, :], fv_ps[:], bvb_sb[:],
                                            Alu.add)
                    nc.vector.memset(fvz[:, 1, :], 0.0)

                def p2(w):
                    """Pass 2: logits, segment softmax, weighted segment sum."""
                    s = st[w]
                    ohw, fvz = s["ohw"], s["fvz"]
                    wsum_ps = wsum_ps_pool.tile(
                        [P, D + 1], f32, name=f"ws{rep}_{w}", tag="wsum_ps")
                    e_win = ew_pool.tile([P, T_W], f32, name=f"ew{rep}_{w}",
                                         tag="e_win")
                    z_win = ew_pool.tile([P, T_W], f32, name=f"zw{rep}_{w}",
                                         tag="z_win")
                    ohT = {}

                    def emit_trb_batch(k):
                        # transpose tiles 4k..4k+3 into one PSUM bank, one copy
                        trb = trb_ps_pool.tile([P, 4, P], bf16,
                                               name=f"trb{rep}_{w}_{k}",
                                               tag="trb")
                        for j in range(4):
                            nc.tensor.transpose(trb[:, j, :],
                                                ohw[:, 4 * k + j, :], idb_sb[:])
                        oh4 = ohT_pool.tile([P, 4, P], fp8,
                                            name=f"ohT{rep}_{w}_{k}", tag="ohT")
                        if k % 2 == 0:
                            nc.vector.tensor_copy(oh4[:], trb[:])
                        else:
                            nc.scalar.copy(oh4[:], trb[:])
                        ohT[k] = oh4

                    emit_trb_batch(0)
                    n_wsum = 0  # tiles whose ohz+wsum have been emitted

                    def emit_wsum_tile():
                        nonlocal n_wsum
                        t = n_wsum
                        ohz = ohz_pool.tile([P, P], bf16,
                                            name=f"ohz{rep}_{w}_{t}", tag="ohz")
                        nc.vector.tensor_scalar(ohz[:], ohw[:, t, :],
                                                z_win[:, t:t + 1], None,
                                                Alu.mult)
                        nc.tensor.matmul(wsum_ps[:], ohz[:], s["nat"](t),
                                         start=(t == 0), stop=(t == T_W - 1))
                        n_wsum += 1

                    def emit_z_chunk(c):
                        # z = exp(e) = sigmoid(e)/sigmoid(-e): stays on the
                        # sigmoid ACT table (no table swaps).
                        c0, c1 = c * CHZ, (c + 1) * CHZ
                        sp = zch_pool.tile([P, CHZ], f32,
                                           name=f"sp{rep}_{w}_{c}", tag="zch")
                        nc.scalar.activation(sp[:], e_win[:, c0:c1],
                                             Act.Sigmoid)
                        sn = zch_pool.tile([P, CHZ], f32,
                                           name=f"sn{rep}_{w}_{c}", tag="zch")
                        nc.scalar.activation(sn[:], e_win[:, c0:c1],
                                             Act.Sigmoid, scale=-1.0)
                        rn = zch_pool.tile([P, CHZ], f32,
                                           name=f"rn{rep}_{w}_{c}", tag="zch")
                        nc.vector.reciprocal(rn[:], sn[:])
                        nc.vector.tensor_tensor(z_win[:, c0:c1], sp[:],
                                                rn[:], Alu.mult)

                    s_ps = None
                    zc = 0  # z chunks emitted
                    for t in range(T_W):
                        if t % 4 == 0 and t + 4 < T_W:
                            emit_trb_batch(t // 4 + 1)
                        if t % 2 == 0:
                            s_ps = s_ps_pool.tile([P, 2 * D], f32,
                                                  name=f"sps{rep}_{w}_{t}",
                                                  tag="s_ps")
                        sl = slice((t % 2) * D, (t % 2) * D + D)
                        nc.tensor.matmul(s_ps[:, sl], s["ifT"](t), wu8_sb[:],
                                         start=True, stop=False, perf_mode=DR)
                        k, j = t // 4, t % 4
                        if j < 3:
                            nc.tensor.matmul(s_ps[:, sl], ohT[k][:, j:j + 2, :],
                                             fvz[:], start=False, stop=True,
                                             perf_mode=DR)
                        else:
                            nc.tensor.matmul(s_ps[:, sl], ohT[k][:, 3, :],
                                             fvz[:, 0, :], start=False,
                                             stop=True)
                        if t % 2 == 1:
                            s_sb = s_pool.tile([P, 2 * D], bf16,
                                               name=f"ssb{rep}_{w}_{t}",
                                               tag="s_sb")
                            nc.scalar.activation(s_sb[:], s_ps[:], Act.Sigmoid)
                            for tt in (t - 1, t):
                                ssl = slice((tt % 2) * D, (tt % 2) * D + D)
                                prod = prod_pool.tile(
                                    [P, D], bf16, name=f"pr{rep}_{w}_{tt}",
                                    tag="prod")
                                nc.vector.scalar_tensor_tensor(
                                    out=prod[:], in0=s_sb[:, ssl], scalar=1.0,
                                    in1=web_sb[:], op0=Alu.mult, op1=Alu.mult,
                                    accum_out=e_win[:, tt:tt + 1])
                        # emit z chunks whose e columns are all accumulated
                        stt_done = t + 1 if t % 2 == 1 else t - 1
                        while zc < 4 and (zc + 1) * CHZ <= stt_done:
                            emit_z_chunk(zc)
                            zc += 1
                        # trail the logits stage with z-gated weighted sums
                        if n_wsum < zc * CHZ and n_wsum <= t - CHZ:
                            emit_wsum_tile()
                        yield
                    while n_wsum < T_W:
                        emit_wsum_tile()
                        if n_wsum % 4 == 0:
                            yield

                    den = col_pool.tile([P, 1], f32, name=f"den{rep}_{w}",
                                        tag="col")
                    nc.vector.tensor_scalar(den[:], wsum_ps[:, D:D + 1], 1e-30,
                                            None, Alu.max)
                    rden = col_pool.tile([P, 1], f32, name=f"rden{rep}_{w}",
                                         tag="col")
                    nc.vector.reciprocal(rden[:], den[:])
                    out_sb = s["out_sb"]
                    nc.vector.tensor_scalar(out_sb[:, 0:D], wsum_ps[:, 0:D],
                                            rden[:], None, Alu.mult)
                    nc.sync.dma_start(out_dram[w], out_sb[:])

                # software pipeline: p1(0); [p2(w) x p1(w+1)]; p2 tail
                for _ in p1(0):
                    pass
                for w in range(W_PER_CORE):
                    mid(w)
                    nxt = p1(w + 1) if w + 1 < W_PER_CORE else None
                    for _ in p2(w):
                        if nxt is not None:
                            next(nxt, None)
                    if nxt is not None:
                        for _ in nxt:
                            pass

            if loop_repeat is not None:
                with tc.For_i(0, loop_repeat, 1):
                    body("L")
            else:
                for rep in range(repeat):
                    body(rep)

    return nc


def _prepare(ifeat, Wu, Wv, bv, we, seg_ids):
    """Host-side shard + pad + layout. Returns (T_W, in_maps)."""
    ifeat = np.asarray(ifeat, dtype=np.float32)
    Wu = np.asarray(Wu, dtype=np.float32)
    Wv = np.asarray(Wv, dtype=np.float32)
    bv = np.asarray(bv, dtype=np.float32)
    we = np.asarray(we, dtype=np.float32)
    seg_ids = np.asarray(seg_ids)

    bounds = np.searchsorted(
        seg_ids, np.arange(0, B + 1, SEGS_PER_WINDOW), side="left")
    n_w = np.diff(bounds)
    T_W = max(4, int(-(-int(n_w.max()) // P)))
    T_W = ((T_W + 3) // 4) * 4
    NT = W_PER_CORE * T_W

    wu8 = np.ascontiguousarray(Wu.T).reshape(2, P, D).astype(F8)
    wvT = np.ascontiguousarray(Wv.T).reshape(2, P, D).astype(BF)
    web = np.tile(we, (P, 1)).astype(BF)
    bvb = np.tile(bv, (P, 1)).astype(np.float32)
    idb = np.eye(P, dtype=BF)
    iota = np.tile(np.arange(P, dtype=np.float32), (P, 1)).astype(BF)

    in_maps = []
    for c in range(N_CORES):
        nat = np.zeros((NT * P, D + 1), dtype=np.float32)
        nat[:, D] = 1.0
        seg = np.full((NT * P,), 500.0, dtype=np.float32)
        for wl in range(W_PER_CORE):
            w = c * W_PER_CORE + wl
            lo, hi = bounds[w], bounds[w + 1]
            base = wl * T_W * P
            nat[base:base + (hi - lo), 0:D] = ifeat[lo:hi]
            seg[base:base + (hi - lo)] = (
                seg_ids[lo:hi].astype(np.float32) - w * SEGS_PER_WINDOW)
        natb = nat.astype(BF).reshape(NT, P, D + 1)
        natp = np.ascontiguousarray(natb.transpose(1, 0, 2))      # [P, NT, 257]
        x = nat[:, 0:D].astype(F8).reshape(NT, P, 2, P)           # [g,i,kb,d]
        iftp = np.ascontiguousarray(x.transpose(3, 0, 2, 1))      # [d, g, kb, i]
        segp = np.ascontiguousarray(seg.reshape(NT, P).T)         # [P, NT]
        in_maps.append({
            "natp": natp, "iftp": iftp, "segp": segp,
            "wu8": wu8, "wvT": wvT, "web": web, "bvb": bvb,
            "idb": idb, "iota": iota,
        })
    return T_W, in_maps


_LAST = {}


def _run(ifeat, Wu, Wv, bv, we, seg_ids, trace=False):
    from concourse.bass_utils import run_bass_kernel_spmd

    T_W, in_maps = _prepare(ifeat, Wu, Wv, bv, we, seg_ids)
    nc = _build(T_W)
    _split_sync_waits(nc)
    res = run_bass_kernel_spmd(nc, in_maps, list(range(N_CORES)), trace=trace)
    _LAST["res"] = res
    _LAST["T_W"] = T_W
    _LAST["nc"] = nc
    _LAST["in_maps"] = in_maps

    out = np.empty((B, 2 * D), dtype=np.float32)
    for c in range(N_CORES):
        core_out = res.results[c]["out"]  # [W_PER_CORE, P, 2D]
        for wl in range(W_PER_CORE):
            w = c * W_PER_CORE + wl
            out[w * SEGS_PER_WINDOW:(w + 1) * SEGS_PER_WINDOW, :] = core_out[wl]
    return out


def kernel(ifeat, Wu, Wv, bv, we, seg_ids):
    return _run(ifeat, Wu, Wv, bv, we, seg_ids, trace=False)
